# revision 27
# baseline (speedup 1.0000x reference)
"""Trainium2 Bass kernel for the LoRA dynamics MLP.

Math: out = L2(relu(L1(relu(L0(concat(state, action))))))
with Li(x) = x @ (Wi + s*Ui@Di).T + bi  (LoRA folded into the base GEMM,
exact algebra: x@W.T + s*(x@Di.T)@Ui.T == x@(W + s*Ui@Di).T).

Distribution: pure data parallel over 8 NeuronCores (batch 65536 -> 8192
rows/core); the ~1 MB of folded weights are replicated.

Device layout: activations are feature-major ([features, batch] -- features
on SBUF partitions), so every layer is a plain
psum[mj] = sum_k WT[k, mj-slice].T @ xT[k, :] accumulation; L0/L1 bias+ReLU
are fused on the ScalarE activation (PSUM -> SBUF), L2 bias-add runs on the
DVE. All matmul operands are fp16 (full PE rate, 1 cycle/row; ~3e-4
quantization rms per tensor) accumulated in fp32 PSUM, which halves HBM
traffic vs fp32 (DMA ~79us at the modeled ~360 GB/s vs 102.4us of PE).

The batch-tile loop is software-pipelined on the PE: iteration i runs
L0(i) -> L1(i-1) -> L2(i-2), so the ScalarE activation latency between
layers hides under the next tile's matmuls and the PE never stalls in
steady state. Host does layout only (transpose/concat/shard/cast) plus the
tiny O(H*R*F) LoRA fold in float64.
"""

import numpy as np

import concourse.mybir as mybir
import concourse.tile as tile
from concourse import bacc, bass_utils

P = 128
B = 65536
S = 768
A = 128
F0 = S + A            # 896
H = 256
NCORES = 8
BC = B // NCORES      # 8192 rows per core
BT = 512              # batch tile (matmul moving dim; one f32 PSUM bank)
NBT = BC // BT        # 16 batch tiles per core
KO0, KO1, KO2 = F0 // P, H // P, H // P     # 7, 2, 2 contraction tiles
MO0, MO1, MO2 = H // P, H // P, S // P      # 2, 2, 6 output tiles
LORA_SCALE = 16.0 / 8.0

F32 = mybir.dt.float32
F16 = mybir.dt.float16
RELU = mybir.ActivationFunctionType.Relu
IDENT = mybir.ActivationFunctionType.Identity

_NC_CACHE = []
LAST_RESULT = None  # BassKernelResults of the most recent run (for test.py)
LAST_BOUNDS = [2, 4, 6]  # y-store chunk boundaries for the final tile


def _build(xp_bufs=4, hp_bufs=2, pp_bufs=8, op_bufs=3, in_split=2, y_split=3,
           wu_n=40, wu_ap=64, tail_split=False):
    nc = bacc.Bacc("TRN2", target_bir_lowering=False, debug=False,
                   num_devices=NCORES)
    xT = nc.dram_tensor("xT", [F0, BC], F16, kind="ExternalInput").ap()
    w0t = nc.dram_tensor("w0t", [F0, H], F16, kind="ExternalInput").ap()
    w1t = nc.dram_tensor("w1t", [H, H], F16, kind="ExternalInput").ap()
    w2t = nc.dram_tensor("w2t", [H, S], F16, kind="ExternalInput").ap()
    b0 = nc.dram_tensor("b0", [H], F32, kind="ExternalInput").ap()
    b1 = nc.dram_tensor("b1", [H], F32, kind="ExternalInput").ap()
    b2 = nc.dram_tensor("b2", [S], F32, kind="ExternalInput").ap()
    yT = nc.dram_tensor("yT", [S, BC], F16, kind="ExternalOutput").ap()

    w0t_t = w0t.rearrange("(ko p) m -> p ko m", p=P)
    xT_t = xT.rearrange("(ko p) b -> p ko b", p=P)
    yT_t = yT.rearrange("(mo p) b -> p mo b", p=P)

    # k-chunk boundaries for the startup split of w0 / x(0): the first
    # matmuls need only low k, so interleaving w0/x0 chunks lets the PE
    # start ~2us earlier than a monolithic load.
    kb = [round(g * KO0 / in_split) for g in range(in_split + 1)]

    with tile.TileContext(nc) as tc:
        with (
            tc.tile_pool(name="wp", bufs=1) as wp,
            tc.tile_pool(name="xp", bufs=xp_bufs) as xp,
            tc.tile_pool(name="h1p", bufs=hp_bufs) as h1p,
            tc.tile_pool(name="h2p", bufs=hp_bufs) as h2p,
            tc.tile_pool(name="pp", bufs=pp_bufs, space="PSUM") as pp,
            tc.tile_pool(name="op", bufs=op_bufs) as op,
        ):
            w0_sb = wp.tile([P, KO0, H], F16)
            b0_sb = wp.tile([P, MO0], F32)
            w1_sb = wp.tile([P, KO1, H], F16)
            w2_sb = wp.tile([P, KO2, S], F16)
            b1_sb = wp.tile([P, MO1], F32)
            b2_sb = wp.tile([P, MO2], F32)

            x_tiles = {}
            h1_tiles = {}
            h2_tiles = {}

            # work items: (col0, width, last_use_of_x_tile). Tiles 0..NBT-2
            # are full 512-wide; the final tile is split into two 256-wide
            # halves so the drain->store->sem tail chain at the very end is
            # half as deep.
            if tail_split:
                items = [(t * BT, BT) for t in range(NBT - 1)]
                items += [((NBT - 1) * BT, BT // 2),
                          ((NBT - 1) * BT + BT // 2, BT // 2)]
            else:
                items = [(t * BT, BT) for t in range(NBT)]
            NIT = len(items)

            def load_x(t, split=1):
                x_sb = xp.tile([P, KO0, BT], F16, tag="x")
                bsl = slice(t * BT, (t + 1) * BT)
                if split == 1:
                    nc.sync.dma_start(x_sb[:], xT_t[:, :, bsl])
                else:
                    for g in range(split):
                        ks = slice(kb[g], kb[g + 1])
                        nc.sync.dma_start(x_sb[:, ks, :], xT_t[:, ks, bsl])
                x_tiles[t] = x_sb

            def s0(i):
                # L0 matmuls + fused bias+ReLU -> h1(i)
                col0, w = items[i]
                t, off = col0 // BT, col0 % BT
                x_sb = x_tiles[t]
                if off + w == BT:
                    x_tiles.pop(t)
                h1 = h1p.tile([P, KO1, w], F16, tag="h1")
                for mj in range(MO0):
                    ps = pp.tile([P, w], F32, tag="ps")
                    for k in range(KO0):
                        nc.tensor.matmul(ps[:], w0_sb[:, k, mj * P:(mj + 1) * P],
                                         x_sb[:, k, off:off + w],
                                         start=(k == 0), stop=(k == KO0 - 1))
                    nc.scalar.activation(h1[:, mj, :], ps[:], RELU,
                                         bias=b0_sb[:, mj:mj + 1], scale=1.0)
                h1_tiles[i] = h1

            def s1(i):
                # L1 matmuls + fused bias+ReLU -> h2(i)
                _, w = items[i]
                h1 = h1_tiles.pop(i)
                h2 = h2p.tile([P, KO2, w], F16, tag="h2")
                for mj in range(MO1):
                    ps = pp.tile([P, w], F32, tag="ps")
                    for k in range(KO1):
                        nc.tensor.matmul(ps[:], w1_sb[:, k, mj * P:(mj + 1) * P],
                                         h1[:, k, :],
                                         start=(k == 0), stop=(k == KO1 - 1))
                    nc.scalar.activation(h2[:, mj, :], ps[:], RELU,
                                         bias=b1_sb[:, mj:mj + 1], scale=1.0)
                h2_tiles[i] = h2

            def s2(i):
                # L2 matmuls + bias-add, alternating DVE / ScalarE (Identity
                # with bias; Identity and Relu share every activation table,
                # so no table reloads) -> y(i), stored in chunks so the drain
                # overlaps the output DMA.
                col0, w = items[i]
                h2 = h2_tiles.pop(i)
                bsl = slice(col0, col0 + w)
                o_sb = op.tile([P, MO2, w], F16, tag="o")
                if i == NIT - 1:
                    # finer chunks at the very end: the last store is on the
                    # critical path (drain -> DMA -> sem), so keep it small
                    bounds = LAST_BOUNDS
                else:
                    bounds = [MO2 * (g + 1) // y_split for g in range(y_split)]
                # near the end the ScalarE still owes L1 activations, so give
                # it fewer drains there to get the stores out sooner
                dve_mjs = {0, 1, 2, 4} if i == NIT - 2 else {0, 2, 4}
                for mj in range(MO2):
                    ps = pp.tile([P, w], F32, tag="ps")
                    for k in range(KO2):
                        nc.tensor.matmul(ps[:], w2_sb[:, k, mj * P:(mj + 1) * P],
                                         h2[:, k, :],
                                         start=(k == 0), stop=(k == KO2 - 1))
                    if mj in dve_mjs:
                        nc.vector.tensor_tensor(
                            o_sb[:, mj, :], ps[:],
                            b2_sb[:, mj:mj + 1].to_broadcast((P, w)),
                            mybir.AluOpType.add)
                    else:
                        nc.scalar.activation(o_sb[:, mj, :], ps[:], IDENT,
                                             bias=b2_sb[:, mj:mj + 1],
                                             scale=1.0)
                    if (mj + 1) in bounds:
                        gi = bounds.index(mj + 1)
                        lo = 0 if gi == 0 else bounds[gi - 1]
                        msl = slice(lo, mj + 1)
                        nc.sync.dma_start(yT_t[:, msl, bsl], o_sb[:, msl, :])

            # -- PE warm-up: the cost model ramps the PE clock (0.65 GHz ->
            # 1.2 GHz -> 2.4 GHz over ~3us of continuous execution). Run tiny
            # matmuls on memset data while the first DMAs are in flight so
            # the ramp finishes before the real matmuls start.
            if wu_n:
                wu = wp.tile([P, P + wu_ap], F16, tag="wu")
                nc.vector.memset(wu[:], 0.0)
                wu_ps = pp.tile([P, BT], F32, tag="ps")
                for _ in range(wu_n):
                    nc.tensor.matmul(wu_ps[:, :wu_ap], wu[:, :P],
                                     wu[:, P:P + wu_ap], start=True, stop=True)

            # -- prologue: interleave w0/x0 k-chunks so the PE starts early;
            # x(1)/x(2) go ahead of the weights the PE won't need for a while
            # (every DMA serializes on the HWDGE + DMA-engine devices).
            x0_sb = xp.tile([P, KO0, BT], F16, tag="x")
            for g in range(in_split):
                ks = slice(kb[g], kb[g + 1])
                nc.sync.dma_start(w0_sb[:, ks, :], w0t_t[:, ks, :])
                nc.sync.dma_start(x0_sb[:, ks, :], xT_t[:, ks, 0:BT])
            x_tiles[0] = x0_sb
            load_x(1, split=2)
            nc.sync.dma_start(b0_sb[:], b0.rearrange("(mo p) -> p mo", p=P))
            load_x(2)
            nc.sync.dma_start(w1_sb[:], w1t.rearrange("(ko p) m -> p ko m", p=P))
            nc.sync.dma_start(w2_sb[:], w2t.rearrange("(ko p) m -> p ko m", p=P))
            nc.sync.dma_start(b1_sb[:], b1.rearrange("(mo p) -> p mo", p=P))
            nc.sync.dma_start(b2_sb[:], b2.rearrange("(mo p) -> p mo", p=P))

            # -- software-pipelined main loop --
            loaded = {0, 1, 2}
            for i in range(NIT + 2):
                if i + 3 <= NIT - 1:
                    t_need = items[i + 3][0] // BT
                    if t_need not in loaded:
                        loaded.add(t_need)
                        load_x(t_need)
                if i <= NIT - 1:
                    s0(i)
                if 0 <= i - 1 <= NIT - 1:
                    s1(i - 1)
                if 0 <= i - 2 <= NIT - 1:
                    s2(i - 2)
    nc.compile()
    return nc


def kernel(state, action, W0, b0, W1, b1, W2, b2,
           D0, U0, D1, U1, D2, U2):
    global LAST_RESULT
    state = np.asarray(state, dtype=np.float32)
    action = np.asarray(action, dtype=np.float32)

    def fold(W, U, D):
        # exact LoRA merge, done in float64 to keep the fold itself lossless
        We = W.astype(np.float64) + LORA_SCALE * (
            U.astype(np.float64) @ D.astype(np.float64))
        return np.ascontiguousarray(We.T.astype(np.float16))  # [in, out]

    w0t = fold(np.asarray(W0), np.asarray(U0), np.asarray(D0))
    w1t = fold(np.asarray(W1), np.asarray(U1), np.asarray(D1))
    w2t = fold(np.asarray(W2), np.asarray(U2), np.asarray(D2))
    b0 = np.ascontiguousarray(np.asarray(b0, dtype=np.float32))
    b1 = np.ascontiguousarray(np.asarray(b1, dtype=np.float32))
    b2 = np.ascontiguousarray(np.asarray(b2, dtype=np.float32))

    # feature-major input, sharded over cores along batch
    xT = np.empty((F0, B), dtype=np.float16)
    xT[:S] = state.T
    xT[S:] = action.T

    if not _NC_CACHE:
        _NC_CACHE.append(_build())
    nc = _NC_CACHE[0]

    in_maps = [
        {
            "xT": np.ascontiguousarray(xT[:, c * BC:(c + 1) * BC]),
            "w0t": w0t, "w1t": w1t, "w2t": w2t,
            "b0": b0, "b1": b1, "b2": b2,
        }
        for c in range(NCORES)
    ]
    res = bass_utils.run_bass_kernel_spmd(nc, in_maps,
                                          core_ids=list(range(NCORES)))
    LAST_RESULT = res

    out = np.empty((B, S), dtype=np.float32)
    for c in range(NCORES):
        out[c * BC:(c + 1) * BC, :] = res.results[c]["yT"].T.astype(np.float32)
    return out
